# revision 50
# baseline (speedup 1.0000x reference)
"""PointNet (2x PointNetConv + global max pool + linear head) on 8 TRN2 cores.

Data-parallel over the 8 graphs: core c owns graph c (1024 nodes, 16-NN).
Per-core pipeline entirely on-chip:
  kNN via f16 hi/lo matmul + f32 vector.max/max_index/match_replace
  (top-16), software-pipelined at chunk granularity with the conv1
  gathers: neighbor indices are replicated/transposed fully on-chip
  (broadcast scalar copy + PE transpose, no DRAM bounce), so chunk m's
  dma_gather desc-gen starts right after chunk m's top-k.
  conv1/conv2 via linearity: h[e] = T[j_e] - C(pos_i); T gathered with
  gpsimd dma_gather (hi/lo f16 table, 256B/edge = hw minimum) and the
  -C(pos_i) term folded into a second accumulating matmul with a
  stride-0-broadcast rhs over pos, so the gather-phase fold costs the
  vector engine nothing; full-population BN stats AllReduced across
  cores (any stat subsetting measurably hurts: the per-node C/A terms
  cluster, so effective sample count is ~nodes, not edges).
  conv2 L3 + per-graph max pool fused as an edge-global max: [128,1024]
  PSUM blocks max-reduced on vector (1 col/cycle is the hard floor; f16
  reduce is no faster, scalar can't reduce, Pool can't read PSUM or
  reduce along free dims). The [B,latent] head AllGathers in quarters
  so only the last 2-oc collective is exposed.
All pre-BN biases cancel (BN absorbs constant shifts) and are dropped.
"""

import os

import numpy as np

import concourse.bacc as bacc
import concourse.mybir as mybir
import concourse.tile as tile
from concourse import bass_utils
from concourse.masks import make_identity

dt = mybir.dt
F32, F16, I16, U16 = dt.float32, dt.float16, dt.int16, dt.uint16
AF = mybir.ActivationFunctionType
ALU = mybir.AluOpType
AX = mybir.AxisListType

B = 8
N = 1024
K = 16
E = N * K  # 16384 edges per core
NCORES = 8
EPS = 1e-5
RG = [list(range(NCORES))]
# chunks (of 32) contributing to BN stats; rest overlap the CC
SSTAT = int(os.environ.get("KSSTAT", "32"))
T_F16D = os.environ.get("KF16D", "0") == "1"   # f16 top-k score matrix
T_MM2 = os.environ.get("KMM2", "1") == "1"     # -C via accumulating matmul
T_X1HI = os.environ.get("KX1HI", "1") == "1"   # x1 f16-only into G2
_STAGE = int(os.environ.get("KSTAGE", "99"))


class _Stop(Exception):
    pass


def _emit(nc, tc, d, out_d):
    # ---------------- persistent tiles ----------------
    # Keep free fns alive: dropping them GC-releases the pool mid-trace,
    # letting later pools reuse the same SBUF zone (AddressConflictError).
    frees = []

    def T(shape, dtype, name):
        ap, f = tc.tile(shape, dtype, name=name)
        frees.append(f)
        return ap

    ident = T([128, 128], F32, "ident")
    fold = T([128, 64], F16, "fold")
    posT_sb = T([3, N], F32, "posT_sb")
    pos16 = T([3, N], F16, "pos16")
    poslo = T([3, N], F16, "poslo")
    pos9 = T([9, N], F16, "pos9")  # [p16; plo; p16] for the -C matmuls
    rhs11 = T([11, N], F16, "rhs11")
    lhsT11 = T([11, N], F16, "lhsT11")
    p2hi = T([3, N], F16, "p2hi")
    p2lo = T([3, N], F16, "p2lo")
    sqhi16 = T([1, N], F16, "sqhi16")
    sqlo16 = T([1, N], F16, "sqlo16")
    sqpos = T([1, N], F32, "sqpos")
    possq = T([3, N], F32, "possq")
    possqh = T([3, N], F16, "possqh")
    possql = T([3, N], F16, "possql")
    ones3f = T([3, 1], F16, "ones3f")
    w1sum_sb = T([3, 64], F32, "w1sum_sb")
    w1b9_sb = T([9, 64], F16, "w1b9_sb")
    w21p9_sb = T([9, 64], F16, "w21p9_sb")
    w1b_sb = T([3, 64], F32, "w1b_sb")
    w21p_sb = T([3, 64], F32, "w21p_sb")
    C1_sb = T([64, N], F32, "C1_sb")
    C2_sb = T([64, N], F32, "C2_sb")
    w2_sb = T([64, 64], F16, "w2_sb")
    w2s = T([64, 64], F16, "w2s")
    w22s = T([64, 128], F16, "w22s")
    w3s = T([128, 1024], F16, "w3s")
    w21x_sb = T([64, 64], F16, "w21x_sb")
    w21p16_sb = T([3, 64], F16, "w21p16_sb")
    w22_sb = T([64, 128], F16, "w22_sb")
    w3_sb = T([128, 1024], F16, "w3_sb")
    linw_sb = T([128, 2048], F32, "linw_sb")  # [p, kc*256 + m]
    gb1_sb = T([64, 2], F32, "gb1_sb")
    gb21_sb = T([64, 2], F32, "gb21_sb")
    gb22_sb = T([128, 2], F32, "gb22_sb")
    gbf_sb = T([128, 4], F32, "gbf_sb")
    packed32 = T([128, 128], dt.uint32, "packed32")
    pf128 = T([128, 128], F32, "pf128")
    idxw = [T([128, 128], I16, f"idxw{m}") for m in range(8)]
    A_sb = T([64, N], F32, "A_sb")
    G2_sb = T([64, N], F32, "G2_sb")
    NT_hi = T([128, 512], F16, "NT_hi")  # [p, c0*64 + ch]
    NT_lo = T([128, 512], F16, "NT_lo")
    x1m = T([64, N], F32, "x1m")
    x1hi = T([64, N], F16, "x1hi")
    x1lo = T([64, N], F16, "x1lo")
    gats = [T([128, 512], F16, f"gat{c}") for c in range(32)]
    gc8 = T([128, 1024], F16, "gc8")  # [p, (oc*16+eb)*8 + j]
    g_sb = T([128, 8], F32, "g_sb")
    go_sb = T([8, 1024], F32, "go_sb")
    gall_sb = T([128, 64], F32, "gall_sb")  # [p, kc*8 + graph]
    bnst = T([128, 192], F32, "bnst")  # 32 blocks x 6
    csum = T([64, 32], F32, "csum")  # per-chunk scalar-accumulated sums
    csq = T([64, 32], F32, "csq")
    qd512 = T([64, 512], F16, "qd512")  # discard target for square pass
    st2 = T([128, 2], F32, "st2")
    st_in = T([128, 2], F32, "st_in")
    st_g = T([128, 2], F32, "st_g")
    bnw = T([128, 10], F32, "bnw")
    qdump = T([128, 8], F32, "qdump")
    outf_sb = T([128, 8], F32, "outf_sb")
    osb = T([8, 128], F32, "osb")
    v8a = T([128, 8], F16 if T_F16D else F32, "v8a")
    v8b = T([128, 8], F16 if T_F16D else F32, "v8b")
    epsc = T([128, 1], F32, "epsc")
    idxd = T([16, 8], I16, "idxd")
    hA = T([128, E], F16, "hA")
    hB = T([128, E], F16, "hB")

    def ck(stage):
        if _STAGE <= stage:
            raise _Stop

    try:
        with tc.tile_pool(name="scr", bufs=3) as scr, \
             tc.tile_pool(name="psb", bufs=3, space="PSUM") as ps_big, \
             tc.tile_pool(name="pss", bufs=2, space="PSUM") as ps_small, \
             tc.tile_pool(name="dtab", bufs=1, space="DRAM") as dtab, \
             tc.tile_pool(name="dcc", bufs=1, space="DRAM") as dcc:

            a_tab = dtab.tile([N, 128], F16, name="a_tab")
            g2_tab = dtab.tile([N, 128], F16, name="g2_tab")
            dum_tab = dtab.tile([128, 128], F16, name="dum_tab")
            ccw_i = dcc.tile([8, 1], F32, name="ccw_i")
            ccw_o = dcc.tile([8, 1], F32, name="ccw_o")
            cc1i = dcc.tile([64, 2], F32, name="cc1i")
            cc1o = dcc.tile([64, 2], F32, name="cc1o")
            cc2i = dcc.tile([64, 2], F32, name="cc2i")
            cc2o = dcc.tile([64, 2], F32, name="cc2o")
            cc3i = dcc.tile([128, 2], F32, name="cc3i")
            cc3o = dcc.tile([128, 2], F32, name="cc3o")
            g_inq = [dcc.tile([256], F32, name=f"g_in{q}") for q in range(4)]
            g_outq = [dcc.tile([B, 256], F32, name=f"g_out{q}")
                      for q in range(4)]

            # ---------------- load weights ----------------
            # only what the kNN critical path + build_ac need up front; the
            # bulky later-stage weights load after the pos-prep DMAs
            nc.sync.dma_start(out=posT_sb[:], in_=d["posT"][:])
            nc.sync.dma_start(out=w1sum_sb[:], in_=d["w1sum"][:])
            nc.sync.dma_start(out=w1b9_sb[:], in_=d["w1b9"][:])
            nc.sync.dma_start(out=w21p9_sb[:], in_=d["w21p9"][:])
            nc.sync.dma_start(out=w1b_sb[:], in_=d["w1b"][:])
            nc.sync.dma_start(out=w21p_sb[:], in_=d["w21p"][:])

            # ident/fold come from the host: building them on-chip needs the
            # gpsimd affine_select ext-isa library, and the later library
            # swap to the dma_gather ucode inserts a quiesce barrier that
            # waits for ALL in-flight DMAs (~30us stall before the first
            # gather). With no affine use, the single gather-library load is
            # forced right here by a 128-idx dummy gather while the DMA
            # pipeline is still nearly empty.
            nc.sync.dma_start(out=ident[:], in_=d["identw"][:])
            nc.sync.dma_start(out=fold[:], in_=d["foldw"][:])
            nc.vector.memset(epsc[:], EPS)
            nc.vector.memset(idxd[:], 0)
            nc.gpsimd.dma_gather(
                out_ap=gats[0][:, 0:128].rearrange("p (one e) -> p one e",
                                                   one=1),
                in_ap=dum_tab[:], idxs_ap=idxd[:], num_idxs=128,
                num_idxs_reg=128, elem_size=128, transpose=True, queue_num=0)
            # No warmup collective: the kernel-start ncfw barrier (~31us on
            # the CC cores) already does the heavy init, so the first real
            # AllReduce costs ~12us either way -- and a warmup issued early
            # would block the Pool FIFO (and thus the first gathers) behind
            # that barrier.

            # ---------------- pos prep ----------------
            # Score matrix in f16 hi/lo (one 11-row f16 matmul per half
            # instead of fp32 LOW/HIGH double-pass: ~4x less PE time):
            # s = 2(phi+plo).(phi'+plo') - (sqhi+sqlo), dropping plo.plo'.
            # |p|^2 via two accumulating f16 matmuls over the hi/lo split of
            # p^2 -- no fp32 LOW_HIGH pass, no DMA-assembled rhs, and every
            # operand stays at partition base 0.
            nc.vector.tensor_copy(pos16[:], posT_sb[:])
            nc.scalar.square(possq[:], posT_sb[:])
            nc.scalar.copy(possqh[:], possq[:])
            nc.vector.tensor_tensor(out=poslo[:], in0=posT_sb[:], in1=pos16[:],
                                    op=ALU.subtract)
            nc.vector.tensor_tensor(out=possql[:], in0=possq[:],
                                    in1=possqh[:], op=ALU.subtract)
            nc.vector.memset(ones3f[:], 1.0)
            for h in range(2):
                sl = slice(h * 512, (h + 1) * 512)
                sq_ps = ps_small.tile([1, 512], F32, name="small")
                nc.tensor.matmul(out=sq_ps[:], lhsT=ones3f[:],
                                 rhs=possqh[:, sl], start=True, stop=False)
                nc.tensor.matmul(out=sq_ps[:], lhsT=ones3f[:],
                                 rhs=possql[:, sl], start=False, stop=True)
                nc.scalar.copy(sqpos[:, sl], sq_ps[:])
            nc.scalar.copy(sqhi16[:], sqpos[:])
            nc.vector.tensor_tensor(out=sqlo16[:], in0=sqpos[:], in1=sqhi16[:],
                                    op=ALU.subtract)
            nc.scalar.mul(p2hi[:], pos16[:], 2.0)
            nc.scalar.mul(p2lo[:], poslo[:], 2.0)
            nc.vector.memset(lhsT11[:], -1.0)
            nc.sync.dma_start(out=lhsT11[0:3, :], in_=p2hi[:])
            nc.sync.dma_start(out=lhsT11[3:6, :], in_=p2hi[:])
            nc.sync.dma_start(out=lhsT11[6:9, :], in_=p2lo[:])
            nc.sync.dma_start(out=rhs11[0:3, :], in_=pos16[:])
            nc.sync.dma_start(out=rhs11[3:6, :], in_=poslo[:])
            nc.sync.dma_start(out=rhs11[6:9, :], in_=pos16[:])
            nc.sync.dma_start(out=rhs11[9:10, :], in_=sqhi16[:])
            nc.sync.dma_start(out=rhs11[10:11, :], in_=sqlo16[:])
            nc.sync.dma_start(out=pos9[0:3, :], in_=pos16[:])
            nc.sync.dma_start(out=pos9[3:6, :], in_=poslo[:])
            nc.sync.dma_start(out=pos9[6:9, :], in_=pos16[:])

            nc.sync.dma_start(out=w2_sb[:], in_=d["w2w"][:])
            nc.sync.dma_start(out=w21x_sb[:], in_=d["w21x"][:])
            nc.sync.dma_start(out=w21p16_sb[:], in_=d["w21p16"][:])
            nc.sync.dma_start(out=w22_sb[:], in_=d["w22w"][:])
            nc.sync.dma_start(out=w3_sb[:], in_=d["w3w"][:])
            nc.sync.dma_start(
                out=linw_sb[:].rearrange("p (kc m) -> p kc m", m=256),
                in_=d["linw"].rearrange("(kc p) m -> p kc m", p=128),
            )
            nc.sync.dma_start(out=gb1_sb[:], in_=d["gb1"][:])
            nc.sync.dma_start(out=gb21_sb[:], in_=d["gb21"][:])
            nc.sync.dma_start(out=gb22_sb[:], in_=d["gb22"][:])
            nc.sync.dma_start(out=gbf_sb[:], in_=d["gbf"][:])
            ck(1)

            # ---------------- A node features + a_tab ----------
            # built at kNN chunk 0 so the conv1 gathers (which need a_tab AND
            # the first idxw chunk) can start right after chunk 0's top-k.
            def build_ac(dst, lhsT_w, nm):
                ps = ps_big.tile([64, N], F32, name="big")
                for h in range(2):
                    sl = slice(h * 512, (h + 1) * 512)
                    nc.tensor.matmul(out=ps[:, sl], lhsT=lhsT_w[:],
                                     rhs=posT_sb[:, sl], start=True, stop=True)
                nc.scalar.copy(dst[:], ps[:])

            def build_table(src_sb, tab, c0s, lo=True):
                # tab[j, 0:64] = f16 hi of src[:, j]; tab[j, 64:128] = f16 lo
                # (or zeros when lo=False -- the 256B gather elem is the hw
                # minimum, so the lo half rides along for free either way)
                for c0 in c0s:
                    pT = ps_small.tile([128, 64], F32, name="small")
                    nc.tensor.transpose(pT[:], src_sb[:, c0 * 128:(c0 + 1) * 128],
                                        ident[0:64, 0:64])
                    hi = NT_hi[:, c0 * 64:(c0 + 1) * 64]
                    nc.scalar.copy(hi, pT[:])
                    if lo:
                        nc.vector.tensor_tensor(
                            out=NT_lo[:, c0 * 64:(c0 + 1) * 64],
                            in0=pT[:], in1=hi, op=ALU.subtract)
                tabv = tab.rearrange("(c p) ch -> p c ch", p=128)
                hiv = NT_hi[:].rearrange("p (c ch) -> p c ch", ch=64)
                lov = NT_lo[:].rearrange("p (c ch) -> p c ch", ch=64)
                c0a, c0b = c0s[0], c0s[-1] + 1
                nc.sync.dma_start(out=tabv[:, c0a:c0b, 0:64],
                                  in_=hiv[:, c0a:c0b])
                nc.sync.dma_start(out=tabv[:, c0a:c0b, 64:128],
                                  in_=lov[:, c0a:c0b])

            ck(3)

            # ---------------- kNN top-16 ----------------
            # score[i,j] = 2 p_i.p_j - |p_j|^2  (row-constant -|p_i|^2 dropped)
            # idxw is built per-CHUNK (128 nodes) so conv1 gathers for chunk m
            # start while chunk m+1's top-k runs.
            def knn_chunk(m):
                D_ps = ps_big.tile([128, N], F32, name="big")
                for h in range(2):
                    sl = slice(h * 512, (h + 1) * 512)
                    nc.tensor.matmul(out=D_ps[:, sl],
                                     lhsT=lhsT11[:, m * 128:(m + 1) * 128],
                                     rhs=rhs11[:, sl], start=True, stop=True)
                D_sb = scr.tile([128, N], F16 if T_F16D else F32, name="dsb")
                nc.scalar.copy(D_sb[:], D_ps[:])
                nc.vector.max(v8a[:], D_sb[:])
                nc.vector.max_index(packed32[:, m * 16:m * 16 + 8], v8a[:], D_sb[:])
                nc.vector.match_replace(D_sb[:], v8a[:], D_sb[:],
                                        -60000.0 if T_F16D else -1e30)
                nc.vector.max(v8b[:], D_sb[:])
                nc.vector.max_index(packed32[:, m * 16 + 8:m * 16 + 16], v8b[:],
                                    D_sb[:])
                # idxw build fully on-chip: one broadcast scalar copy
                # replicates the 16 idx cols 8x (u32 -> f32 exact), one PE
                # transpose flips [node, g*16+k] -> [g*16+k, node], one scalar
                # copy converts to i16 -- no SP DMAs, no DRAM bounce (the old
                # 9-DMA-per-chunk bounce saturated the SP engine and starved
                # the gather queues).
                cs = slice(m * 16, (m + 1) * 16)
                nc.scalar.copy(
                    pf128[:].rearrange("p (g k) -> p g k", g=8),
                    packed32[:, cs].bitcast(mybir.dt.int32)
                    .rearrange("p (one k) -> p one k", one=1)
                    .to_broadcast([128, 8, 16]))
                pT2 = ps_small.tile([128, 128], F32, name="small")
                nc.tensor.transpose(pT2[:], pf128[:], ident[:])
                nc.scalar.copy(idxw[m][:], pT2[:])
            ck(2)

            def fold_one(et, h_t, w9_sb, c_sb, stats_now):
                # h[:, chunk] = (hi+lo fold of gathered T) - w9^T pos_i, both
                # as PE matmuls accumulating into one PSUM tile; the scalar
                # engine copies PSUM -> f16 SBUF with accum_out giving the
                # per-chunk BN sum, plus a square pass for E[x^2] -- the
                # vector engine does NO stats work in the gather phases (it
                # is saturated by the f32 top-k feeding the gathers).
                g = gats[et]
                sl = slice(et * 512, (et + 1) * 512)
                s_ps = ps_small.tile([64, 512], F32, name="small")
                if T_MM2:
                    nc.tensor.matmul(out=s_ps[:], lhsT=fold[:], rhs=g[:],
                                     start=True, stop=False)
                    nc.tensor.matmul(
                        out=s_ps[:], lhsT=w9_sb[:],
                        rhs=pos9[:, et * 32:(et + 1) * 32].to_broadcast(
                            [9, 32, 16]),
                        start=False, stop=True)
                    nc.scalar.copy(h_t[:, sl], s_ps[:])
                else:
                    nc.tensor.matmul(out=s_ps[:], lhsT=fold[:], rhs=g[:],
                                     start=True, stop=True)
                    nc.vector.tensor_tensor(
                        out=h_t[:, sl].rearrange("c (i k) -> c i k", k=16),
                        in0=s_ps[:].rearrange("c (i k) -> c i k", k=16),
                        in1=c_sb[:, et * 32:(et + 1) * 32].to_broadcast(
                            [64, 32, 16]),
                        op=ALU.subtract)
                # full-population BN stats: ANY subsetting (chunk- or
                # edge-strided) measurably hurts accuracy because the
                # reference mean/var are exact and errors compound over 3 BN
                # layers; interleaves with the gather stream on vector.
                if stats_now:
                    nc.vector.bn_stats(bnst[0:64, et * 6:(et + 1) * 6],
                                       h_t[:, sl])

            def gather_chunk(m, tab, h_t, w9_sb, c_sb, stats_now=True):
                # hw limit: <=512 idxs per dma_gather call; per-chunk tiles so
                # chunk c+1's DMA overlaps chunk c's fold matmul
                for q in range(4):
                    et = m * 4 + q
                    g = gats[et]
                    nc.gpsimd.dma_gather(
                        out_ap=g[:].rearrange("p (one e) -> p one e", one=1),
                        in_ap=tab[:],
                        idxs_ap=idxw[m][:, q * 32:(q + 1) * 32],
                        num_idxs=512,
                        num_idxs_reg=512, elem_size=128, transpose=True,
                        queue_num=et % 4)
                    fold_one(et, h_t, w9_sb, c_sb, stats_now)

            def bn_relu(h_t, out_t, P, gb_sb, cci, cco, sc_mod=4):
                # global-batch BN (AllReduce mean/E[x^2]), then relu' =
                # relu(x + b/a) split vector-heavy (f16 tensor_scalar runs in
                # the DVE fast mode, ~2.4x the scalar ACT rate); the a-scale
                # folds into the NEXT layer's weights (a = gamma/sigma > 0)
                nc.vector.bn_aggr(st2[0:P, :], bnst[0:P, 0:6 * SSTAT])
                nc.scalar.copy(st_in[0:P, 0:1], st2[0:P, 0:1])
                nc.scalar.square(st_in[0:P, 1:2], st2[0:P, 0:1])
                nc.vector.tensor_tensor(out=st_in[0:P, 1:2],
                                        in0=st_in[0:P, 1:2],
                                        in1=st2[0:P, 1:2], op=ALU.add)
                nc.sync.dma_start(out=cci[:], in_=st_in[0:P, :])
                nc.gpsimd.collective_compute("AllReduce", ALU.add,
                                             replica_groups=RG,
                                             ins=[cci.opt()], outs=[cco.opt()])
                nc.sync.dma_start(out=st_g[0:P, :], in_=cco[:])
                _bn_coeffs(P, gb_sb)
                nc.vector.reciprocal(bnw[0:P, 8:9], bnw[0:P, 6:7])
                nc.vector.tensor_tensor(out=bnw[0:P, 8:9], in0=bnw[0:P, 7:8],
                                        in1=bnw[0:P, 8:9], op=ALU.mult)  # b/a
                for c in range(8):
                    sl = slice(c * 2048, (c + 1) * 2048)
                    if c % sc_mod == sc_mod - 1:
                        nc.scalar.activation(out_t[:, sl], h_t[:, sl], AF.Relu,
                                             bias=bnw[0:P, 8:9], scale=1.0)
                    else:
                        nc.vector.tensor_scalar(
                            out=out_t[:, sl], in0=h_t[:, sl],
                            scalar1=bnw[0:P, 8:9], scalar2=0.0,
                            op0=ALU.add, op1=ALU.max)

            def scale_rows(dst, src, P):
                # dst = diag(a) @ src, folding the BN scale into the next
                # layer's stationary weights
                nc.vector.tensor_tensor(
                    out=dst[:], in0=src[:],
                    in1=bnw[0:P, 6:7].to_broadcast(list(src.shape)),
                    op=ALU.mult)

            def _bn_coeffs(P, gb_sb):
                nc.scalar.mul(bnw[0:P, 0:1], st_g[0:P, 0:1], 1.0 / NCORES)  # m
                nc.scalar.mul(bnw[0:P, 1:2], st_g[0:P, 1:2], 1.0 / NCORES)  # q
                nc.scalar.square(bnw[0:P, 2:3], bnw[0:P, 0:1])
                nc.vector.tensor_tensor(out=bnw[0:P, 3:4], in0=bnw[0:P, 1:2],
                                        in1=bnw[0:P, 2:3], op=ALU.subtract)  # var
                nc.scalar.activation(bnw[0:P, 4:5], bnw[0:P, 3:4], AF.Sqrt,
                                     bias=epsc[0:P, 0:1], scale=1.0)
                nc.vector.reciprocal(bnw[0:P, 5:6], bnw[0:P, 4:5])
                nc.vector.tensor_tensor(out=bnw[0:P, 6:7], in0=gb_sb[0:P, 0:1],
                                        in1=bnw[0:P, 5:6], op=ALU.mult)  # scale
                nc.vector.tensor_tensor(out=bnw[0:P, 8:9], in0=bnw[0:P, 0:1],
                                        in1=bnw[0:P, 6:7], op=ALU.mult)
                nc.vector.tensor_tensor(out=bnw[0:P, 7:8], in0=gb_sb[0:P, 1:2],
                                        in1=bnw[0:P, 8:9], op=ALU.subtract)  # bias

            # ---------------- conv1 ----------------
            # software-pipelined with the kNN: per-engine FIFOs are strictly
            # program-ordered, so the chunk-m fold matmuls are emitted only
            # two chunks behind chunk-m+2's D matmul / idx transpose -- the
            # PE never FIFO-blocks on a not-yet-computed top-k, and gather
            # desc-gen starts ~8us after the first top-k. conv1 bn_stats are
            # emitted AFTER all top-k vector work so they don't slow the
            # top-k cadence that feeds the gather queues.
            h1 = hA[0:64, :]
            # a_tab is built before the kNN chunks: it needs only posT, and
            # the first gather waits on it -- built inline it was the last
            # arrival (~42us) gating the whole conv1 gather stream.
            build_ac(A_sb, w1sum_sb, "A_ps")
            if not T_MM2:
                build_ac(C1_sb, w1b_sb, "C1_ps")
                build_ac(C2_sb, w21p_sb, "C2_ps")
            build_table(A_sb, a_tab, range(8))
            for m in range(10):
                if m < 8:
                    knn_chunk(m)
                if m >= 2:
                    gather_chunk(m - 2, a_tab, h1, w1b9_sb, C1_sb,
                                 stats_now=False)
            for et in range(32):
                nc.vector.bn_stats(bnst[0:64, et * 6:(et + 1) * 6],
                                   h1[:, et * 512:(et + 1) * 512])
            ck(5)
            relu1 = hB[0:64, :]
            bn_relu(h1, relu1, 64, gb1_sb, cc1i, cc1o, sc_mod=2)
            scale_rows(w2s, w2_sb, 64)
            ck(6)
            # x1m pairs packed into one [128,512] PSUM tile (odd chunk at
            # partition base 64 via tile_position) -- the k-max reduce then
            # uses all 128 DVE lanes, halving the reduce instructions on the
            # post-bn1 critical path; scalar unpacks straight into f16 x1hi.
            for pr in range(16):
                ps = ps_small.tile([128, 512], F32, name="small")
                for half in range(2):
                    et = pr * 2 + half
                    sl = slice(et * 512, (et + 1) * 512)
                    nc.tensor.matmul(out=ps[half * 64:(half + 1) * 64, :],
                                     lhsT=w2s[:], rhs=relu1[:, sl],
                                     start=True, stop=True,
                                     skip_group_check=True)
                x1p = scr.tile([128, 32], F32, name="x1p")
                nc.vector.tensor_reduce(
                    out=x1p[:],
                    in_=ps[:].rearrange("c (i k) -> c i k", k=16),
                    axis=AX.X, op=ALU.max)
                for half in range(2):
                    et = pr * 2 + half
                    nc.scalar.copy(x1hi[:, et * 32:(et + 1) * 32],
                                   x1p[half * 64:(half + 1) * 64, :])

            ck(7)
            # ---------------- conv2 ----------------
            # x1 enters G2 in plain f16 (BN-normalized next, so the ~5e-4
            # relative quantization is lost in the noise); the table itself
            # keeps the hi/lo split.
            # g2 table is f16-hi only (zero lo): G2 is BN-normalized right
            # after, so the ~5e-4 quantization is immaterial, and the table
            # build drops the per-block lo subtract + can go per-half so the
            # first conv2 gathers start after half 0.
            nc.vector.memset(NT_lo[:], 0.0)
            G2_ps = ps_big.tile([64, N], F32, name="big")
            for h in range(2):
                sl = slice(h * 512, (h + 1) * 512)
                nc.tensor.matmul(out=G2_ps[:, sl], lhsT=w21x_sb[:],
                                 rhs=x1hi[:, sl], start=True, stop=False)
                nc.tensor.matmul(out=G2_ps[:, sl], lhsT=w21p16_sb[:],
                                 rhs=pos16[:, sl], start=False, stop=False)
                nc.tensor.matmul(out=G2_ps[:, sl], lhsT=w21p16_sb[:],
                                 rhs=poslo[:, sl], start=False, stop=True)
                nc.scalar.copy(G2_sb[:, sl], G2_ps[:, sl])
                build_table(G2_sb, g2_tab, range(4 * h, 4 * h + 4), lo=False)
            ck(8)

            h21 = hA[0:64, :]
            for m in range(8):
                gather_chunk(m, g2_tab, h21, w21p9_sb, C2_sb)
            relu21 = hB[0:64, :]
            bn_relu(h21, relu21, 64, gb21_sb, cc2i, cc2o, sc_mod=2)

            scale_rows(w22s, w22_sb, 64)
            h22 = hA[:]
            for et in range(32):
                sl = slice(et * 512, (et + 1) * 512)
                ps = ps_small.tile([128, 512], F32, name="small")
                nc.tensor.matmul(out=ps[:], lhsT=w22s[:], rhs=relu21[:, sl],
                                 start=True, stop=True)
                nc.scalar.copy(h22[:, sl], ps[:])
                if et < SSTAT:
                    nc.vector.bn_stats(bnst[:, et * 6:(et + 1) * 6],
                                       h22[:, sl])
            relu22 = hB[:]
            bn_relu(h22, relu22, 128, gb22_sb, cc3i, cc3o, sc_mod=4)
            scale_rows(w3s, w3_sb, 128)
            ck(9)

            # ---------------- conv2 L3 + edge-max pool ----------------
            # [128,1024] 2-bank PSUM chunks, max-reduced straight from PSUM.
            # The g AllGather + final-linear accumulation are quartered so
            # each 2-oc AllGather overlaps the remaining L3 matmuls and only
            # the last quarter's collective latency is exposed.
            def ag_issue(q2):
                # reduce + kick the AllGather for oc pair q2
                for oc in range(q2 * 2, q2 * 2 + 2):
                    nc.vector.tensor_reduce(
                        out=g_sb[:, oc:oc + 1],
                        in_=gc8[:, oc * 128:(oc + 1) * 128],
                        axis=AX.X, op=ALU.max)
                nc.sync.dma_start(
                    out=g_inq[q2].rearrange("(c p) -> p c", p=128),
                    in_=g_sb[:, q2 * 2:(q2 + 1) * 2])
                nc.gpsimd.collective_compute("AllGather", ALU.bypass,
                                             replica_groups=RG,
                                             ins=[g_inq[q2].opt()],
                                             outs=[g_outq[q2].opt()])
                nc.sync.dma_start(out=go_sb[:, q2 * 256:(q2 + 1) * 256],
                                  in_=g_outq[q2][:])

            def ag_consume(q2):
                for cl in range(2):
                    c = q2 * 2 + cl
                    pT = ps_big.tile([128, 8], F32, name="big")
                    nc.tensor.transpose(
                        pT[:], go_sb[:, c * 128:(c + 1) * 128],
                        ident[0:8, 0:8])
                    nc.scalar.copy(gall_sb[:, c * 8:(c + 1) * 8], pT[:])

            for oc in range(8):
                for eb in range(16):
                    ps = ps_big.tile([128, 1024], F32, name="big")
                    for h in range(2):
                        sl = slice(eb * 1024 + h * 512,
                                   eb * 1024 + (h + 1) * 512)
                        nc.tensor.matmul(out=ps[:, h * 512:(h + 1) * 512],
                                         lhsT=w3s[:, oc * 128:(oc + 1) * 128],
                                         rhs=relu22[:, sl],
                                         start=True, stop=True)
                    col = oc * 16 + eb
                    # direct f32 PSUM max-reduce: the DVE streams 1 col/cycle
                    # regardless of dtype or source (f16 SBUF measured the
                    # same 1.2us), scalar cannot reduce, and the Pool engine
                    # only reduces along partitions -- this 1024-col read per
                    # eb block is the hard floor of the L3 phase.
                    nc.vector.tensor_reduce(
                        out=gc8[:, col * 8:(col + 1) * 8],
                        in_=ps[:].rearrange("p (j e) -> p j e", j=8),
                        axis=AX.X, op=ALU.max)
                if oc % 2 == 1 and oc < 7:
                    ag_issue(oc // 2)

            ck(10)
            ag_issue(3)
            psfs = []
            for oc2 in range(2):
                psf = ps_small.tile([128, 8], F32, name="small")
                psfs.append(psf)
            for q2 in range(4):
                ag_consume(q2)
                for oc2 in range(2):
                    psf = psfs[oc2]
                    for kc in range(q2 * 2, q2 * 2 + 2):
                        base = kc * 256 + oc2 * 128
                        nc.tensor.matmul(out=psf[:],
                                         lhsT=linw_sb[:, base:base + 128],
                                         rhs=gall_sb[:, kc * 8:(kc + 1) * 8],
                                         start=(kc == 0), stop=(kc == 7),
                                         skip_group_check=True)

            ck(11)
            # ---------------- final linear + local BN + relu ----------------
            for oc2 in range(2):
                psf = psfs[oc2]
                nc.vector.tensor_reduce(out=bnw[:, 9:10], in_=psf[:],
                                        axis=AX.X, op=ALU.add)
                nc.scalar.mul(bnw[:, 0:1], bnw[:, 9:10], 1.0 / B)  # m
                nc.scalar.activation(qdump[:], psf[:], AF.Square,
                                     accum_out=bnw[:, 9:10])
                nc.scalar.mul(bnw[:, 1:2], bnw[:, 9:10], 1.0 / B)  # q
                nc.scalar.square(bnw[:, 2:3], bnw[:, 0:1])
                nc.vector.tensor_tensor(out=bnw[:, 3:4], in0=bnw[:, 1:2],
                                        in1=bnw[:, 2:3], op=ALU.subtract)
                nc.scalar.activation(bnw[:, 4:5], bnw[:, 3:4], AF.Sqrt,
                                     bias=epsc[:], scale=1.0)
                nc.vector.reciprocal(bnw[:, 5:6], bnw[:, 4:5])
                nc.vector.tensor_tensor(out=bnw[:, 6:7],
                                        in0=gbf_sb[:, oc2:oc2 + 1],
                                        in1=bnw[:, 5:6], op=ALU.mult)
                nc.vector.tensor_tensor(out=bnw[:, 8:9], in0=bnw[:, 0:1],
                                        in1=bnw[:, 6:7], op=ALU.mult)
                nc.vector.tensor_tensor(out=bnw[:, 7:8],
                                        in0=gbf_sb[:, 2 + oc2:3 + oc2],
                                        in1=bnw[:, 8:9], op=ALU.subtract)
                nc.scalar.activation(outf_sb[:], psf[:], AF.Relu,
                                     bias=bnw[:, 7:8], scale=bnw[:, 6:7])
                pso = ps_big.tile([8, 128], F32, name="big")
                nc.tensor.transpose(pso[:], outf_sb[:], ident[:])
                nc.scalar.copy(osb[:], pso[:])
                nc.sync.dma_start(out=out_d[:, oc2 * 128:(oc2 + 1) * 128],
                                  in_=osb[:])

    except _Stop:
        pass
    for f in reversed(frees):
        f()


def _build():
    nc = bacc.Bacc("TRN2", target_bir_lowering=False, debug=False,
                   num_devices=NCORES, num_swdge_queues=4)
    d = {}

    def inp(name, shape, dtype):
        d[name] = nc.dram_tensor(name, shape, dtype, kind="ExternalInput").ap()

    inp("posT", [3, N], F32)
    inp("identw", [128, 128], F32)
    inp("foldw", [128, 64], F16)
    inp("w1sum", [3, 64], F32)
    inp("w1b9", [9, 64], F16)
    inp("w21p9", [9, 64], F16)
    inp("w1b", [3, 64], F32)
    inp("w21p", [3, 64], F32)
    inp("w2w", [64, 64], F16)
    inp("w21x", [64, 64], F16)
    inp("w21p16", [3, 64], F16)
    inp("w22w", [64, 128], F16)
    inp("w3w", [128, 1024], F16)
    inp("linw", [1024, 256], F32)
    inp("gb1", [64, 2], F32)
    inp("gb21", [64, 2], F32)
    inp("gb22", [128, 2], F32)
    inp("gbf", [128, 4], F32)
    out_d = nc.dram_tensor("out", [B, 256], F32, kind="ExternalOutput").ap()

    with tile.TileContext(nc) as tc:
        _emit(nc, tc, d, out_d)
    nc.finalize()
    return nc


_NC = None


def _get_nc():
    global _NC
    if _NC is None:
        _NC = _build()
    return _NC


def _hilo9(w):
    # [-w_hi; -w_hi; -w_lo] pairing with pos9 = [p_hi; p_lo; p_hi]:
    # recovers w_hi*p_hi + w_hi*p_lo + w_lo*p_hi (full product minus the
    # negligible w_lo*p_lo cross term).
    f16, f32 = np.float16, np.float32
    wn = (-w).astype(f32)
    hi = wn.astype(f16)
    lo = (wn - hi.astype(f32)).astype(f16)
    return np.ascontiguousarray(np.vstack([hi, hi, lo]))


def _prepare_in_maps(inputs):
    f32 = np.float32
    f16 = np.float16
    pos = np.asarray(inputs["pos"], dtype=f32)
    c1_W1 = np.asarray(inputs["c1_W1"], dtype=f32)
    c2_W1 = np.asarray(inputs["c2_W1"], dtype=f32)
    common = {
        "identw": np.eye(128, dtype=f32),
        "foldw": np.ascontiguousarray(
            np.vstack([np.eye(64), np.eye(64)]).astype(f16)),
        "w1sum": np.ascontiguousarray(c1_W1[0:3] + c1_W1[3:6]),
        "w1b9": _hilo9(c1_W1[3:6]),
        "w21p9": _hilo9(c2_W1[64:67]),
        "w1b": np.ascontiguousarray(c1_W1[3:6]),
        "w21p": np.ascontiguousarray(c2_W1[64:67]),
        "w2w": np.asarray(inputs["c1_W2"], dtype=f16),
        "w21x": np.ascontiguousarray(c2_W1[0:64].astype(f16)),
        "w21p16": np.ascontiguousarray(c2_W1[64:67].astype(f16)),
        "w22w": np.asarray(inputs["c2_W2"], dtype=f16),
        "w3w": np.asarray(inputs["c2_W3"], dtype=f16),
        "linw": np.asarray(inputs["lin_W"], dtype=f32),
        "gb1": np.ascontiguousarray(
            np.stack([inputs["c1_g1"], inputs["c1_be1"]], axis=1).astype(f32)),
        "gb21": np.ascontiguousarray(
            np.stack([inputs["c2_g1"], inputs["c2_be1"]], axis=1).astype(f32)),
        "gb22": np.ascontiguousarray(
            np.stack([inputs["c2_g2"], inputs["c2_be2"]], axis=1).astype(f32)),
        "gbf": np.ascontiguousarray(np.stack(
            [np.asarray(inputs["lin_g"], dtype=f32)[0:128],
             np.asarray(inputs["lin_g"], dtype=f32)[128:256],
             np.asarray(inputs["lin_be"], dtype=f32)[0:128],
             np.asarray(inputs["lin_be"], dtype=f32)[128:256]], axis=1)),
    }
    in_maps = []
    for c in range(NCORES):
        m = dict(common)
        m["posT"] = np.ascontiguousarray(pos[c * N:(c + 1) * N].T)
        in_maps.append(m)
    return in_maps


def _run(inputs, trace=False, **kw):
    return bass_utils.run_bass_kernel_spmd(
        _get_nc(), _prepare_in_maps(inputs),
        core_ids=list(range(NCORES)), trace=trace, **kw)


def kernel(**inputs):
    res = _run(inputs)
    return np.asarray(res.results[0]["out"], dtype=np.float32)


# revision 51
# speedup vs baseline: 1.0193x; 1.0193x over previous
"""PointNet (2x PointNetConv + global max pool + linear head) on 8 TRN2 cores.

Data-parallel over the 8 graphs: core c owns graph c (1024 nodes, 16-NN).
Per-core pipeline entirely on-chip:
  kNN via f16 hi/lo matmul + f32 vector.max/max_index/match_replace
  (top-16), software-pipelined at chunk granularity with the conv1
  gathers: neighbor indices are replicated/transposed fully on-chip
  (broadcast scalar copy + PE transpose, no DRAM bounce), so chunk m's
  dma_gather desc-gen starts right after chunk m's top-k.
  conv1/conv2 via linearity: h[e] = T[j_e] - C(pos_i); T gathered with
  gpsimd dma_gather (hi/lo f16 table, 256B/edge = hw minimum) and the
  -C(pos_i) term folded into a second accumulating matmul with a
  stride-0-broadcast rhs over pos, so the gather-phase fold costs the
  vector engine nothing; full-population BN stats AllReduced across
  cores (any stat subsetting measurably hurts: the per-node C/A terms
  cluster, so effective sample count is ~nodes, not edges).
  conv2 L3 + per-graph max pool fused as an edge-global max: [128,1024]
  PSUM blocks max-reduced on vector (1 col/cycle is the hard floor; f16
  reduce is no faster, scalar can't reduce, Pool can't read PSUM or
  reduce along free dims). The [B,latent] head AllGathers in quarters
  so only the last 2-oc collective is exposed.
All pre-BN biases cancel (BN absorbs constant shifts) and are dropped.
"""

import os

import numpy as np

import concourse.bacc as bacc
import concourse.mybir as mybir
import concourse.tile as tile
from concourse import bass_utils
from concourse.masks import make_identity

dt = mybir.dt
F32, F16, I16, U16 = dt.float32, dt.float16, dt.int16, dt.uint16
AF = mybir.ActivationFunctionType
ALU = mybir.AluOpType
AX = mybir.AxisListType

B = 8
N = 1024
K = 16
E = N * K  # 16384 edges per core
NCORES = 8
EPS = 1e-5
RG = [list(range(NCORES))]
# chunks (of 32) contributing to BN stats; rest overlap the CC
SSTAT = int(os.environ.get("KSSTAT", "32"))
T_F16D = os.environ.get("KF16D", "0") == "1"   # f16 top-k score matrix
T_MM2 = os.environ.get("KMM2", "1") == "1"     # -C via accumulating matmul
T_X1HI = os.environ.get("KX1HI", "1") == "1"   # x1 f16-only into G2
_STAGE = int(os.environ.get("KSTAGE", "99"))


class _Stop(Exception):
    pass


def _emit(nc, tc, d, out_d):
    # ---------------- persistent tiles ----------------
    # Keep free fns alive: dropping them GC-releases the pool mid-trace,
    # letting later pools reuse the same SBUF zone (AddressConflictError).
    frees = []

    def T(shape, dtype, name):
        ap, f = tc.tile(shape, dtype, name=name)
        frees.append(f)
        return ap

    ident = T([128, 128], F32, "ident")
    fold = T([128, 64], F16, "fold")
    posT_sb = T([3, N], F32, "posT_sb")
    pos16 = T([3, N], F16, "pos16")
    poslo = T([3, N], F16, "poslo")
    pos9 = T([9, N], F16, "pos9")  # [p16; plo; p16] for the -C matmuls
    rhs11 = T([11, N], F16, "rhs11")
    lhsT11 = T([11, N], F16, "lhsT11")
    p2hi = T([3, N], F16, "p2hi")
    p2lo = T([3, N], F16, "p2lo")
    sqhi16 = T([1, N], F16, "sqhi16")
    sqlo16 = T([1, N], F16, "sqlo16")
    sqpos = T([1, N], F32, "sqpos")
    possq = T([3, N], F32, "possq")
    possqh = T([3, N], F16, "possqh")
    possql = T([3, N], F16, "possql")
    ones3f = T([3, 1], F16, "ones3f")
    w1sum_sb = T([3, 64], F32, "w1sum_sb")
    w1b9_sb = T([9, 64], F16, "w1b9_sb")
    w21p9_sb = T([9, 64], F16, "w21p9_sb")
    w1b_sb = T([3, 64], F32, "w1b_sb")
    w21p_sb = T([3, 64], F32, "w21p_sb")
    C1_sb = T([64, N], F32, "C1_sb")
    C2_sb = T([64, N], F32, "C2_sb")
    w2_sb = T([64, 64], F16, "w2_sb")
    w2s = T([64, 64], F16, "w2s")
    w22s = T([64, 128], F16, "w22s")
    w3s = T([128, 1024], F16, "w3s")
    w21x_sb = T([64, 64], F16, "w21x_sb")
    w21p16_sb = T([3, 64], F16, "w21p16_sb")
    w22_sb = T([64, 128], F16, "w22_sb")
    w3_sb = T([128, 1024], F16, "w3_sb")
    linw_sb = T([128, 2048], F32, "linw_sb")  # [p, kc*256 + m]
    gb1_sb = T([64, 2], F32, "gb1_sb")
    gb21_sb = T([64, 2], F32, "gb21_sb")
    gb22_sb = T([128, 2], F32, "gb22_sb")
    gbf_sb = T([128, 4], F32, "gbf_sb")
    packed32 = T([128, 128], dt.uint32, "packed32")
    pf128 = T([128, 128], F32, "pf128")
    idxw = [T([128, 128], I16, f"idxw{m}") for m in range(8)]
    A_sb = T([64, N], F32, "A_sb")
    G2_sb = T([64, N], F32, "G2_sb")
    NT_hi = T([128, 512], F16, "NT_hi")  # [p, c0*64 + ch]
    NT_lo = T([128, 512], F16, "NT_lo")
    x1m = T([64, N], F32, "x1m")
    x1hi = T([64, N], F16, "x1hi")
    x1lo = T([64, N], F16, "x1lo")
    gats = [T([128, 512], F16, f"gat{c}") for c in range(32)]
    gc8 = T([128, 1024], F16, "gc8")  # [p, (oc*16+eb)*8 + j]
    g_sb = T([128, 8], F32, "g_sb")
    go_sb = T([8, 1024], F32, "go_sb")
    gall_sb = T([128, 64], F32, "gall_sb")  # [p, kc*8 + graph]
    bnst = T([128, 192], F32, "bnst")  # 32 blocks x 6
    csum = T([64, 32], F32, "csum")  # per-chunk scalar-accumulated sums
    csq = T([64, 32], F32, "csq")
    qd512 = T([64, 512], F16, "qd512")  # discard target for square pass
    st2 = T([128, 2], F32, "st2")
    st_in = T([128, 2], F32, "st_in")
    st_g = T([128, 2], F32, "st_g")
    bnw = T([128, 10], F32, "bnw")
    qdump = T([128, 8], F32, "qdump")
    outf_sb = T([128, 8], F32, "outf_sb")
    osb = T([8, 128], F32, "osb")
    v8a = T([128, 8], F16 if T_F16D else F32, "v8a")
    v8b = T([128, 8], F16 if T_F16D else F32, "v8b")
    epsc = T([128, 1], F32, "epsc")
    idxd = T([16, 8], I16, "idxd")
    hA = T([128, E], F16, "hA")
    hB = T([128, E], F16, "hB")

    def ck(stage):
        if _STAGE <= stage:
            raise _Stop

    try:
        with tc.tile_pool(name="scr", bufs=3) as scr, \
             tc.tile_pool(name="psb", bufs=3, space="PSUM") as ps_big, \
             tc.tile_pool(name="pss", bufs=2, space="PSUM") as ps_small, \
             tc.tile_pool(name="dtab", bufs=1, space="DRAM") as dtab, \
             tc.tile_pool(name="dcc", bufs=1, space="DRAM") as dcc:

            a_tab = dtab.tile([N, 128], F16, name="a_tab")
            g2_tab = dtab.tile([N, 128], F16, name="g2_tab")
            dum_tab = dtab.tile([128, 128], F16, name="dum_tab")
            ccw_i = dcc.tile([8, 1], F32, name="ccw_i")
            ccw_o = dcc.tile([8, 1], F32, name="ccw_o")
            cc1i = dcc.tile([64, 2], F32, name="cc1i")
            cc1o = dcc.tile([64, 2], F32, name="cc1o")
            cc2i = dcc.tile([64, 2], F32, name="cc2i")
            cc2o = dcc.tile([64, 2], F32, name="cc2o")
            cc3i = dcc.tile([128, 2], F32, name="cc3i")
            cc3o = dcc.tile([128, 2], F32, name="cc3o")
            g_inq = [dcc.tile([256], F32, name=f"g_in{q}") for q in range(4)]
            g_outq = [dcc.tile([B, 256], F32, name=f"g_out{q}")
                      for q in range(4)]

            # ---------------- load weights ----------------
            # only what the kNN critical path + build_ac need up front; the
            # bulky later-stage weights load after the pos-prep DMAs
            nc.sync.dma_start(out=posT_sb[:], in_=d["posT"][:])
            nc.sync.dma_start(out=w1sum_sb[:], in_=d["w1sum"][:])
            nc.sync.dma_start(out=w1b9_sb[:], in_=d["w1b9"][:])
            nc.sync.dma_start(out=w21p9_sb[:], in_=d["w21p9"][:])
            nc.sync.dma_start(out=w1b_sb[:], in_=d["w1b"][:])
            nc.sync.dma_start(out=w21p_sb[:], in_=d["w21p"][:])

            # ident/fold come from the host: building them on-chip needs the
            # gpsimd affine_select ext-isa library, and the later library
            # swap to the dma_gather ucode inserts a quiesce barrier that
            # waits for ALL in-flight DMAs (~30us stall before the first
            # gather). With no affine use, the single gather-library load is
            # forced right here by a 128-idx dummy gather while the DMA
            # pipeline is still nearly empty.
            nc.sync.dma_start(out=ident[:], in_=d["identw"][:])
            nc.sync.dma_start(out=fold[:], in_=d["foldw"][:])
            nc.vector.memset(epsc[:], EPS)
            nc.vector.memset(idxd[:], 0)
            nc.gpsimd.dma_gather(
                out_ap=gats[0][:, 0:128].rearrange("p (one e) -> p one e",
                                                   one=1),
                in_ap=dum_tab[:], idxs_ap=idxd[:], num_idxs=128,
                num_idxs_reg=128, elem_size=128, transpose=True, queue_num=0)
            # No warmup collective: the kernel-start ncfw barrier (~31us on
            # the CC cores) already does the heavy init, so the first real
            # AllReduce costs ~12us either way -- and a warmup issued early
            # would block the Pool FIFO (and thus the first gathers) behind
            # that barrier.

            # ---------------- pos prep ----------------
            # Score matrix in f16 hi/lo (one 11-row f16 matmul per half
            # instead of fp32 LOW/HIGH double-pass: ~4x less PE time):
            # s = 2(phi+plo).(phi'+plo') - (sqhi+sqlo), dropping plo.plo'.
            # |p|^2 via two accumulating f16 matmuls over the hi/lo split of
            # p^2 -- no fp32 LOW_HIGH pass, no DMA-assembled rhs, and every
            # operand stays at partition base 0.
            nc.vector.tensor_copy(pos16[:], posT_sb[:])
            nc.scalar.square(possq[:], posT_sb[:])
            nc.scalar.copy(possqh[:], possq[:])
            nc.vector.tensor_tensor(out=poslo[:], in0=posT_sb[:], in1=pos16[:],
                                    op=ALU.subtract)
            nc.vector.tensor_tensor(out=possql[:], in0=possq[:],
                                    in1=possqh[:], op=ALU.subtract)
            nc.vector.memset(ones3f[:], 1.0)
            for h in range(2):
                sl = slice(h * 512, (h + 1) * 512)
                sq_ps = ps_small.tile([1, 512], F32, name="small")
                nc.tensor.matmul(out=sq_ps[:], lhsT=ones3f[:],
                                 rhs=possqh[:, sl], start=True, stop=False)
                nc.tensor.matmul(out=sq_ps[:], lhsT=ones3f[:],
                                 rhs=possql[:, sl], start=False, stop=True)
                nc.scalar.copy(sqpos[:, sl], sq_ps[:])
            nc.scalar.copy(sqhi16[:], sqpos[:])
            nc.vector.tensor_tensor(out=sqlo16[:], in0=sqpos[:], in1=sqhi16[:],
                                    op=ALU.subtract)
            nc.scalar.mul(p2hi[:], pos16[:], 2.0)
            nc.scalar.mul(p2lo[:], poslo[:], 2.0)
            nc.vector.memset(lhsT11[:], -1.0)
            nc.sync.dma_start(out=lhsT11[0:3, :], in_=p2hi[:])
            nc.sync.dma_start(out=lhsT11[3:6, :], in_=p2hi[:])
            nc.sync.dma_start(out=lhsT11[6:9, :], in_=p2lo[:])
            nc.sync.dma_start(out=rhs11[0:3, :], in_=pos16[:])
            nc.sync.dma_start(out=rhs11[3:6, :], in_=poslo[:])
            nc.sync.dma_start(out=rhs11[6:9, :], in_=pos16[:])
            nc.sync.dma_start(out=rhs11[9:10, :], in_=sqhi16[:])
            nc.sync.dma_start(out=rhs11[10:11, :], in_=sqlo16[:])
            nc.sync.dma_start(out=pos9[0:3, :], in_=pos16[:])
            nc.sync.dma_start(out=pos9[3:6, :], in_=poslo[:])
            nc.sync.dma_start(out=pos9[6:9, :], in_=pos16[:])

            nc.sync.dma_start(out=w2_sb[:], in_=d["w2w"][:])
            nc.sync.dma_start(out=w21x_sb[:], in_=d["w21x"][:])
            nc.sync.dma_start(out=w21p16_sb[:], in_=d["w21p16"][:])
            nc.sync.dma_start(out=w22_sb[:], in_=d["w22w"][:])
            nc.sync.dma_start(out=w3_sb[:], in_=d["w3w"][:])
            nc.sync.dma_start(
                out=linw_sb[:].rearrange("p (kc m) -> p kc m", m=256),
                in_=d["linw"].rearrange("(kc p) m -> p kc m", p=128),
            )
            nc.sync.dma_start(out=gb1_sb[:], in_=d["gb1"][:])
            nc.sync.dma_start(out=gb21_sb[:], in_=d["gb21"][:])
            nc.sync.dma_start(out=gb22_sb[:], in_=d["gb22"][:])
            nc.sync.dma_start(out=gbf_sb[:], in_=d["gbf"][:])
            ck(1)

            # ---------------- A node features + a_tab ----------
            # built at kNN chunk 0 so the conv1 gathers (which need a_tab AND
            # the first idxw chunk) can start right after chunk 0's top-k.
            def build_ac(dst, lhsT_w, nm):
                ps = ps_big.tile([64, N], F32, name="big")
                for h in range(2):
                    sl = slice(h * 512, (h + 1) * 512)
                    nc.tensor.matmul(out=ps[:, sl], lhsT=lhsT_w[:],
                                     rhs=posT_sb[:, sl], start=True, stop=True)
                nc.scalar.copy(dst[:], ps[:])

            def build_table(src_sb, tab, c0s, lo=True):
                # tab[j, 0:64] = f16 hi of src[:, j]; tab[j, 64:128] = f16 lo
                # (or zeros when lo=False -- the 256B gather elem is the hw
                # minimum, so the lo half rides along for free either way)
                for c0 in c0s:
                    pT = ps_small.tile([128, 64], F32, name="small")
                    nc.tensor.transpose(pT[:], src_sb[:, c0 * 128:(c0 + 1) * 128],
                                        ident[0:64, 0:64])
                    hi = NT_hi[:, c0 * 64:(c0 + 1) * 64]
                    nc.scalar.copy(hi, pT[:])
                    if lo:
                        nc.vector.tensor_tensor(
                            out=NT_lo[:, c0 * 64:(c0 + 1) * 64],
                            in0=pT[:], in1=hi, op=ALU.subtract)
                tabv = tab.rearrange("(c p) ch -> p c ch", p=128)
                hiv = NT_hi[:].rearrange("p (c ch) -> p c ch", ch=64)
                lov = NT_lo[:].rearrange("p (c ch) -> p c ch", ch=64)
                c0a, c0b = c0s[0], c0s[-1] + 1
                nc.sync.dma_start(out=tabv[:, c0a:c0b, 0:64],
                                  in_=hiv[:, c0a:c0b])
                nc.sync.dma_start(out=tabv[:, c0a:c0b, 64:128],
                                  in_=lov[:, c0a:c0b])

            ck(3)

            # ---------------- kNN top-16 ----------------
            # score[i,j] = 2 p_i.p_j - |p_j|^2  (row-constant -|p_i|^2 dropped)
            # idxw is built per-CHUNK (128 nodes) so conv1 gathers for chunk m
            # start while chunk m+1's top-k runs.
            def knn_chunk(m):
                D_ps = ps_big.tile([128, N], F32, name="big")
                for h in range(2):
                    sl = slice(h * 512, (h + 1) * 512)
                    nc.tensor.matmul(out=D_ps[:, sl],
                                     lhsT=lhsT11[:, m * 128:(m + 1) * 128],
                                     rhs=rhs11[:, sl], start=True, stop=True)
                D_sb = scr.tile([128, N], F16 if T_F16D else F32, name="dsb")
                nc.scalar.copy(D_sb[:], D_ps[:])
                nc.vector.max(v8a[:], D_sb[:])
                nc.vector.max_index(packed32[:, m * 16:m * 16 + 8], v8a[:], D_sb[:])
                nc.vector.match_replace(D_sb[:], v8a[:], D_sb[:],
                                        -60000.0 if T_F16D else -1e30)
                nc.vector.max(v8b[:], D_sb[:])
                nc.vector.max_index(packed32[:, m * 16 + 8:m * 16 + 16], v8b[:],
                                    D_sb[:])
                # idxw build fully on-chip: one broadcast scalar copy
                # replicates the 16 idx cols 8x (u32 -> f32 exact), one PE
                # transpose flips [node, g*16+k] -> [g*16+k, node], one scalar
                # copy converts to i16 -- no SP DMAs, no DRAM bounce (the old
                # 9-DMA-per-chunk bounce saturated the SP engine and starved
                # the gather queues).
                cs = slice(m * 16, (m + 1) * 16)
                nc.scalar.copy(
                    pf128[:].rearrange("p (g k) -> p g k", g=8),
                    packed32[:, cs].bitcast(mybir.dt.int32)
                    .rearrange("p (one k) -> p one k", one=1)
                    .to_broadcast([128, 8, 16]))
                pT2 = ps_small.tile([128, 128], F32, name="small")
                nc.tensor.transpose(pT2[:], pf128[:], ident[:])
                nc.scalar.copy(idxw[m][:], pT2[:])
            ck(2)

            def fold_one(et, h_t, w9_sb, c_sb, stats_now):
                # h[:, chunk] = (hi+lo fold of gathered T) - w9^T pos_i, both
                # as PE matmuls accumulating into one PSUM tile; the scalar
                # engine copies PSUM -> f16 SBUF with accum_out giving the
                # per-chunk BN sum, plus a square pass for E[x^2] -- the
                # vector engine does NO stats work in the gather phases (it
                # is saturated by the f32 top-k feeding the gathers).
                g = gats[et]
                sl = slice(et * 512, (et + 1) * 512)
                s_ps = ps_small.tile([64, 512], F32, name="small")
                if T_MM2:
                    nc.tensor.matmul(out=s_ps[:], lhsT=fold[:], rhs=g[:],
                                     start=True, stop=False)
                    nc.tensor.matmul(
                        out=s_ps[:], lhsT=w9_sb[:],
                        rhs=pos9[:, et * 32:(et + 1) * 32].to_broadcast(
                            [9, 32, 16]),
                        start=False, stop=True)
                    nc.scalar.copy(h_t[:, sl], s_ps[:])
                else:
                    nc.tensor.matmul(out=s_ps[:], lhsT=fold[:], rhs=g[:],
                                     start=True, stop=True)
                    nc.vector.tensor_tensor(
                        out=h_t[:, sl].rearrange("c (i k) -> c i k", k=16),
                        in0=s_ps[:].rearrange("c (i k) -> c i k", k=16),
                        in1=c_sb[:, et * 32:(et + 1) * 32].to_broadcast(
                            [64, 32, 16]),
                        op=ALU.subtract)
                # full-population BN stats: ANY subsetting (chunk- or
                # edge-strided) measurably hurts accuracy because the
                # reference mean/var are exact and errors compound over 3 BN
                # layers; interleaves with the gather stream on vector.
                if stats_now:
                    nc.vector.bn_stats(bnst[0:64, et * 6:(et + 1) * 6],
                                       h_t[:, sl])

            def gather_chunk(m, tab, h_t, w9_sb, c_sb, stats_now=True):
                # hw limit: <=512 idxs per dma_gather call; per-chunk tiles so
                # chunk c+1's DMA overlaps chunk c's fold matmul
                for q in range(4):
                    et = m * 4 + q
                    g = gats[et]
                    nc.gpsimd.dma_gather(
                        out_ap=g[:].rearrange("p (one e) -> p one e", one=1),
                        in_ap=tab[:],
                        idxs_ap=idxw[m][:, q * 32:(q + 1) * 32],
                        num_idxs=512,
                        num_idxs_reg=512, elem_size=128, transpose=True,
                        queue_num=et % 4)
                    fold_one(et, h_t, w9_sb, c_sb, stats_now)

            def bn_relu(h_t, out_t, P, gb_sb, cci, cco, sc_mod=4):
                # global-batch BN (AllReduce mean/E[x^2]), then relu' =
                # relu(x + b/a) split vector-heavy (f16 tensor_scalar runs in
                # the DVE fast mode, ~2.4x the scalar ACT rate); the a-scale
                # folds into the NEXT layer's weights (a = gamma/sigma > 0)
                nc.vector.bn_aggr(st2[0:P, :], bnst[0:P, 0:6 * SSTAT])
                nc.scalar.copy(st_in[0:P, 0:1], st2[0:P, 0:1])
                nc.scalar.square(st_in[0:P, 1:2], st2[0:P, 0:1])
                nc.vector.tensor_tensor(out=st_in[0:P, 1:2],
                                        in0=st_in[0:P, 1:2],
                                        in1=st2[0:P, 1:2], op=ALU.add)
                nc.sync.dma_start(out=cci[:], in_=st_in[0:P, :])
                nc.gpsimd.collective_compute("AllReduce", ALU.add,
                                             replica_groups=RG,
                                             ins=[cci.opt()], outs=[cco.opt()])
                nc.sync.dma_start(out=st_g[0:P, :], in_=cco[:])
                _bn_coeffs(P, gb_sb)
                nc.vector.reciprocal(bnw[0:P, 8:9], bnw[0:P, 6:7])
                nc.vector.tensor_tensor(out=bnw[0:P, 8:9], in0=bnw[0:P, 7:8],
                                        in1=bnw[0:P, 8:9], op=ALU.mult)  # b/a
                for c in range(8):
                    sl = slice(c * 2048, (c + 1) * 2048)
                    if c % sc_mod == sc_mod - 1:
                        nc.scalar.activation(out_t[:, sl], h_t[:, sl], AF.Relu,
                                             bias=bnw[0:P, 8:9], scale=1.0)
                    else:
                        nc.vector.tensor_scalar(
                            out=out_t[:, sl], in0=h_t[:, sl],
                            scalar1=bnw[0:P, 8:9], scalar2=0.0,
                            op0=ALU.add, op1=ALU.max)

            def scale_rows(dst, src, P):
                # dst = diag(a) @ src, folding the BN scale into the next
                # layer's stationary weights
                nc.vector.tensor_tensor(
                    out=dst[:], in0=src[:],
                    in1=bnw[0:P, 6:7].to_broadcast(list(src.shape)),
                    op=ALU.mult)

            def _bn_coeffs(P, gb_sb):
                nc.scalar.mul(bnw[0:P, 0:1], st_g[0:P, 0:1], 1.0 / NCORES)  # m
                nc.scalar.mul(bnw[0:P, 1:2], st_g[0:P, 1:2], 1.0 / NCORES)  # q
                nc.scalar.square(bnw[0:P, 2:3], bnw[0:P, 0:1])
                nc.vector.tensor_tensor(out=bnw[0:P, 3:4], in0=bnw[0:P, 1:2],
                                        in1=bnw[0:P, 2:3], op=ALU.subtract)  # var
                nc.scalar.activation(bnw[0:P, 4:5], bnw[0:P, 3:4], AF.Sqrt,
                                     bias=epsc[0:P, 0:1], scale=1.0)
                nc.vector.reciprocal(bnw[0:P, 5:6], bnw[0:P, 4:5])
                nc.vector.tensor_tensor(out=bnw[0:P, 6:7], in0=gb_sb[0:P, 0:1],
                                        in1=bnw[0:P, 5:6], op=ALU.mult)  # scale
                nc.vector.tensor_tensor(out=bnw[0:P, 8:9], in0=bnw[0:P, 0:1],
                                        in1=bnw[0:P, 6:7], op=ALU.mult)
                nc.vector.tensor_tensor(out=bnw[0:P, 7:8], in0=gb_sb[0:P, 1:2],
                                        in1=bnw[0:P, 8:9], op=ALU.subtract)  # bias

            # ---------------- conv1 ----------------
            # software-pipelined with the kNN: per-engine FIFOs are strictly
            # program-ordered, so the chunk-m fold matmuls are emitted only
            # two chunks behind chunk-m+2's D matmul / idx transpose -- the
            # PE never FIFO-blocks on a not-yet-computed top-k, and gather
            # desc-gen starts ~8us after the first top-k. conv1 bn_stats are
            # emitted AFTER all top-k vector work so they don't slow the
            # top-k cadence that feeds the gather queues.
            h1 = hA[0:64, :]
            # a_tab is built before the kNN chunks: it needs only posT, and
            # the first gather waits on it -- built inline it was the last
            # arrival (~42us) gating the whole conv1 gather stream.
            build_ac(A_sb, w1sum_sb, "A_ps")
            if not T_MM2:
                build_ac(C1_sb, w1b_sb, "C1_ps")
                build_ac(C2_sb, w21p_sb, "C2_ps")
            build_table(A_sb, a_tab, range(8))
            for m in range(10):
                if m < 8:
                    knn_chunk(m)
                if m == 8:
                    # warmup AllReduce wedged into the Pool FIFO after chunk
                    # 5's gather calls: the kernel-start ncfw barrier (~50us)
                    # has passed by the time Pool reaches it (~57us), its
                    # blocking wait overlaps the top-k-paced tail of the
                    # gather stream, and bn1 then runs on the warm path
                    # (~9us instead of ~21us first-use).
                    nc.sync.dma_start(out=ccw_i[:], in_=epsc[0:8, 0:1])
                    nc.gpsimd.collective_compute(
                        "AllReduce", ALU.add, replica_groups=RG,
                        ins=[ccw_i.opt()], outs=[ccw_o.opt()])
                if m >= 2:
                    gather_chunk(m - 2, a_tab, h1, w1b9_sb, C1_sb,
                                 stats_now=False)
            for et in range(32):
                nc.vector.bn_stats(bnst[0:64, et * 6:(et + 1) * 6],
                                   h1[:, et * 512:(et + 1) * 512])
            ck(5)
            relu1 = hB[0:64, :]
            bn_relu(h1, relu1, 64, gb1_sb, cc1i, cc1o, sc_mod=2)
            scale_rows(w2s, w2_sb, 64)
            ck(6)
            # x1m pairs packed into one [128,512] PSUM tile (odd chunk at
            # partition base 64 via tile_position) -- the k-max reduce then
            # uses all 128 DVE lanes, halving the reduce instructions on the
            # post-bn1 critical path; scalar unpacks straight into f16 x1hi.
            for pr in range(16):
                ps = ps_small.tile([128, 512], F32, name="small")
                for half in range(2):
                    et = pr * 2 + half
                    sl = slice(et * 512, (et + 1) * 512)
                    nc.tensor.matmul(out=ps[half * 64:(half + 1) * 64, :],
                                     lhsT=w2s[:], rhs=relu1[:, sl],
                                     start=True, stop=True,
                                     skip_group_check=True)
                x1p = scr.tile([128, 32], F32, name="x1p")
                nc.vector.tensor_reduce(
                    out=x1p[:],
                    in_=ps[:].rearrange("c (i k) -> c i k", k=16),
                    axis=AX.X, op=ALU.max)
                for half in range(2):
                    et = pr * 2 + half
                    nc.scalar.copy(x1hi[:, et * 32:(et + 1) * 32],
                                   x1p[half * 64:(half + 1) * 64, :])

            ck(7)
            # ---------------- conv2 ----------------
            # x1 enters G2 in plain f16 (BN-normalized next, so the ~5e-4
            # relative quantization is lost in the noise); the table itself
            # keeps the hi/lo split.
            # g2 table is f16-hi only (zero lo): G2 is BN-normalized right
            # after, so the ~5e-4 quantization is immaterial, and the table
            # build drops the per-block lo subtract + can go per-half so the
            # first conv2 gathers start after half 0.
            nc.vector.memset(NT_lo[:], 0.0)
            G2_ps = ps_big.tile([64, N], F32, name="big")
            for h in range(2):
                sl = slice(h * 512, (h + 1) * 512)
                nc.tensor.matmul(out=G2_ps[:, sl], lhsT=w21x_sb[:],
                                 rhs=x1hi[:, sl], start=True, stop=False)
                nc.tensor.matmul(out=G2_ps[:, sl], lhsT=w21p16_sb[:],
                                 rhs=pos16[:, sl], start=False, stop=False)
                nc.tensor.matmul(out=G2_ps[:, sl], lhsT=w21p16_sb[:],
                                 rhs=poslo[:, sl], start=False, stop=True)
                nc.scalar.copy(G2_sb[:, sl], G2_ps[:, sl])
                build_table(G2_sb, g2_tab, range(4 * h, 4 * h + 4), lo=False)
            ck(8)

            h21 = hA[0:64, :]
            for m in range(8):
                gather_chunk(m, g2_tab, h21, w21p9_sb, C2_sb)
            relu21 = hB[0:64, :]
            bn_relu(h21, relu21, 64, gb21_sb, cc2i, cc2o, sc_mod=2)

            scale_rows(w22s, w22_sb, 64)
            h22 = hA[:]
            for et in range(32):
                sl = slice(et * 512, (et + 1) * 512)
                ps = ps_small.tile([128, 512], F32, name="small")
                nc.tensor.matmul(out=ps[:], lhsT=w22s[:], rhs=relu21[:, sl],
                                 start=True, stop=True)
                nc.scalar.copy(h22[:, sl], ps[:])
                if et < SSTAT:
                    nc.vector.bn_stats(bnst[:, et * 6:(et + 1) * 6],
                                       h22[:, sl])
            relu22 = hB[:]
            bn_relu(h22, relu22, 128, gb22_sb, cc3i, cc3o, sc_mod=4)
            scale_rows(w3s, w3_sb, 128)
            ck(9)

            # ---------------- conv2 L3 + edge-max pool ----------------
            # [128,1024] 2-bank PSUM chunks, max-reduced straight from PSUM.
            # The g AllGather + final-linear accumulation are quartered so
            # each 2-oc AllGather overlaps the remaining L3 matmuls and only
            # the last quarter's collective latency is exposed.
            def ag_issue(q2):
                # reduce + kick the AllGather for oc pair q2
                for oc in range(q2 * 2, q2 * 2 + 2):
                    nc.vector.tensor_reduce(
                        out=g_sb[:, oc:oc + 1],
                        in_=gc8[:, oc * 128:(oc + 1) * 128],
                        axis=AX.X, op=ALU.max)
                nc.sync.dma_start(
                    out=g_inq[q2].rearrange("(c p) -> p c", p=128),
                    in_=g_sb[:, q2 * 2:(q2 + 1) * 2])
                nc.gpsimd.collective_compute("AllGather", ALU.bypass,
                                             replica_groups=RG,
                                             ins=[g_inq[q2].opt()],
                                             outs=[g_outq[q2].opt()])
                nc.sync.dma_start(out=go_sb[:, q2 * 256:(q2 + 1) * 256],
                                  in_=g_outq[q2][:])

            def ag_consume(q2):
                for cl in range(2):
                    c = q2 * 2 + cl
                    pT = ps_big.tile([128, 8], F32, name="big")
                    nc.tensor.transpose(
                        pT[:], go_sb[:, c * 128:(c + 1) * 128],
                        ident[0:8, 0:8])
                    nc.scalar.copy(gall_sb[:, c * 8:(c + 1) * 8], pT[:])

            for oc in range(8):
                for eb in range(16):
                    ps = ps_big.tile([128, 1024], F32, name="big")
                    for h in range(2):
                        sl = slice(eb * 1024 + h * 512,
                                   eb * 1024 + (h + 1) * 512)
                        nc.tensor.matmul(out=ps[:, h * 512:(h + 1) * 512],
                                         lhsT=w3s[:, oc * 128:(oc + 1) * 128],
                                         rhs=relu22[:, sl],
                                         start=True, stop=True)
                    col = oc * 16 + eb
                    # direct f32 PSUM max-reduce: the DVE streams 1 col/cycle
                    # regardless of dtype or source (f16 SBUF measured the
                    # same 1.2us), scalar cannot reduce, and the Pool engine
                    # only reduces along partitions -- this 1024-col read per
                    # eb block is the hard floor of the L3 phase.
                    nc.vector.tensor_reduce(
                        out=gc8[:, col * 8:(col + 1) * 8],
                        in_=ps[:].rearrange("p (j e) -> p j e", j=8),
                        axis=AX.X, op=ALU.max)
                if oc % 2 == 1 and oc < 7:
                    ag_issue(oc // 2)

            ck(10)
            ag_issue(3)
            psfs = []
            for oc2 in range(2):
                psf = ps_small.tile([128, 8], F32, name="small")
                psfs.append(psf)
            for q2 in range(4):
                ag_consume(q2)
                for oc2 in range(2):
                    psf = psfs[oc2]
                    for kc in range(q2 * 2, q2 * 2 + 2):
                        base = kc * 256 + oc2 * 128
                        nc.tensor.matmul(out=psf[:],
                                         lhsT=linw_sb[:, base:base + 128],
                                         rhs=gall_sb[:, kc * 8:(kc + 1) * 8],
                                         start=(kc == 0), stop=(kc == 7),
                                         skip_group_check=True)

            ck(11)
            # ---------------- final linear + local BN + relu ----------------
            for oc2 in range(2):
                psf = psfs[oc2]
                nc.vector.tensor_reduce(out=bnw[:, 9:10], in_=psf[:],
                                        axis=AX.X, op=ALU.add)
                nc.scalar.mul(bnw[:, 0:1], bnw[:, 9:10], 1.0 / B)  # m
                nc.scalar.activation(qdump[:], psf[:], AF.Square,
                                     accum_out=bnw[:, 9:10])
                nc.scalar.mul(bnw[:, 1:2], bnw[:, 9:10], 1.0 / B)  # q
                nc.scalar.square(bnw[:, 2:3], bnw[:, 0:1])
                nc.vector.tensor_tensor(out=bnw[:, 3:4], in0=bnw[:, 1:2],
                                        in1=bnw[:, 2:3], op=ALU.subtract)
                nc.scalar.activation(bnw[:, 4:5], bnw[:, 3:4], AF.Sqrt,
                                     bias=epsc[:], scale=1.0)
                nc.vector.reciprocal(bnw[:, 5:6], bnw[:, 4:5])
                nc.vector.tensor_tensor(out=bnw[:, 6:7],
                                        in0=gbf_sb[:, oc2:oc2 + 1],
                                        in1=bnw[:, 5:6], op=ALU.mult)
                nc.vector.tensor_tensor(out=bnw[:, 8:9], in0=bnw[:, 0:1],
                                        in1=bnw[:, 6:7], op=ALU.mult)
                nc.vector.tensor_tensor(out=bnw[:, 7:8],
                                        in0=gbf_sb[:, 2 + oc2:3 + oc2],
                                        in1=bnw[:, 8:9], op=ALU.subtract)
                nc.scalar.activation(outf_sb[:], psf[:], AF.Relu,
                                     bias=bnw[:, 7:8], scale=bnw[:, 6:7])
                pso = ps_big.tile([8, 128], F32, name="big")
                nc.tensor.transpose(pso[:], outf_sb[:], ident[:])
                nc.scalar.copy(osb[:], pso[:])
                nc.sync.dma_start(out=out_d[:, oc2 * 128:(oc2 + 1) * 128],
                                  in_=osb[:])

    except _Stop:
        pass
    for f in reversed(frees):
        f()


def _build():
    nc = bacc.Bacc("TRN2", target_bir_lowering=False, debug=False,
                   num_devices=NCORES, num_swdge_queues=4)
    d = {}

    def inp(name, shape, dtype):
        d[name] = nc.dram_tensor(name, shape, dtype, kind="ExternalInput").ap()

    inp("posT", [3, N], F32)
    inp("identw", [128, 128], F32)
    inp("foldw", [128, 64], F16)
    inp("w1sum", [3, 64], F32)
    inp("w1b9", [9, 64], F16)
    inp("w21p9", [9, 64], F16)
    inp("w1b", [3, 64], F32)
    inp("w21p", [3, 64], F32)
    inp("w2w", [64, 64], F16)
    inp("w21x", [64, 64], F16)
    inp("w21p16", [3, 64], F16)
    inp("w22w", [64, 128], F16)
    inp("w3w", [128, 1024], F16)
    inp("linw", [1024, 256], F32)
    inp("gb1", [64, 2], F32)
    inp("gb21", [64, 2], F32)
    inp("gb22", [128, 2], F32)
    inp("gbf", [128, 4], F32)
    out_d = nc.dram_tensor("out", [B, 256], F32, kind="ExternalOutput").ap()

    with tile.TileContext(nc) as tc:
        _emit(nc, tc, d, out_d)
    nc.finalize()
    return nc


_NC = None


def _get_nc():
    global _NC
    if _NC is None:
        _NC = _build()
    return _NC


def _hilo9(w):
    # [-w_hi; -w_hi; -w_lo] pairing with pos9 = [p_hi; p_lo; p_hi]:
    # recovers w_hi*p_hi + w_hi*p_lo + w_lo*p_hi (full product minus the
    # negligible w_lo*p_lo cross term).
    f16, f32 = np.float16, np.float32
    wn = (-w).astype(f32)
    hi = wn.astype(f16)
    lo = (wn - hi.astype(f32)).astype(f16)
    return np.ascontiguousarray(np.vstack([hi, hi, lo]))


def _prepare_in_maps(inputs):
    f32 = np.float32
    f16 = np.float16
    pos = np.asarray(inputs["pos"], dtype=f32)
    c1_W1 = np.asarray(inputs["c1_W1"], dtype=f32)
    c2_W1 = np.asarray(inputs["c2_W1"], dtype=f32)
    common = {
        "identw": np.eye(128, dtype=f32),
        "foldw": np.ascontiguousarray(
            np.vstack([np.eye(64), np.eye(64)]).astype(f16)),
        "w1sum": np.ascontiguousarray(c1_W1[0:3] + c1_W1[3:6]),
        "w1b9": _hilo9(c1_W1[3:6]),
        "w21p9": _hilo9(c2_W1[64:67]),
        "w1b": np.ascontiguousarray(c1_W1[3:6]),
        "w21p": np.ascontiguousarray(c2_W1[64:67]),
        "w2w": np.asarray(inputs["c1_W2"], dtype=f16),
        "w21x": np.ascontiguousarray(c2_W1[0:64].astype(f16)),
        "w21p16": np.ascontiguousarray(c2_W1[64:67].astype(f16)),
        "w22w": np.asarray(inputs["c2_W2"], dtype=f16),
        "w3w": np.asarray(inputs["c2_W3"], dtype=f16),
        "linw": np.asarray(inputs["lin_W"], dtype=f32),
        "gb1": np.ascontiguousarray(
            np.stack([inputs["c1_g1"], inputs["c1_be1"]], axis=1).astype(f32)),
        "gb21": np.ascontiguousarray(
            np.stack([inputs["c2_g1"], inputs["c2_be1"]], axis=1).astype(f32)),
        "gb22": np.ascontiguousarray(
            np.stack([inputs["c2_g2"], inputs["c2_be2"]], axis=1).astype(f32)),
        "gbf": np.ascontiguousarray(np.stack(
            [np.asarray(inputs["lin_g"], dtype=f32)[0:128],
             np.asarray(inputs["lin_g"], dtype=f32)[128:256],
             np.asarray(inputs["lin_be"], dtype=f32)[0:128],
             np.asarray(inputs["lin_be"], dtype=f32)[128:256]], axis=1)),
    }
    in_maps = []
    for c in range(NCORES):
        m = dict(common)
        m["posT"] = np.ascontiguousarray(pos[c * N:(c + 1) * N].T)
        in_maps.append(m)
    return in_maps


def _run(inputs, trace=False, **kw):
    return bass_utils.run_bass_kernel_spmd(
        _get_nc(), _prepare_in_maps(inputs),
        core_ids=list(range(NCORES)), trace=trace, **kw)


def kernel(**inputs):
    res = _run(inputs)
    return np.asarray(res.results[0]["out"], dtype=np.float32)
